# revision 1
# baseline (speedup 1.0000x reference)
"""Trainium2 Bass kernel for nn_NewDAGExecutor (plan-predictor matmul + 8-step DAG).

Strategy (8 NeuronCores, data-parallel over the 16384 tokens, 2048 tokens/core):
  - Host: transpose each core's token shard to [H, tok], split fp32 into an
    exact fp16 hi/lo pair (lo scaled by 2^11), same for the concatenated
    weight matrix W = [W_init; W_op; W_gate] (168 outputs) and biases.
  - Device: plan = hidden @ W.T + b computed as 3 fp16 matmul passes
    (hi*Whi -> PSUM1, hi*Wlo + lo*Whi -> PSUM2, combine P1 + 2^-11*P2),
    which is fp32-accurate (fp16 products are exact in the fp32 MACs; the
    dropped lo*lo term is ~2^-24 relative). Tokens ride the PSUM partition
    axis so the DAG math lands in token-major layout with no transpose.
  - The 8 sequential DAG steps run wide across all 2048 tokens/core
    ([128 partitions x 16 tile-columns]) on DVE/ACT. tanh(x*1e4) is computed
    via exp to keep every per-step ACT call inside the natural_log_exp table
    set (no per-step table switches); the sign is restored with bitwise ops.
"""

import numpy as np

import concourse.bacc as bacc
import concourse.bass as bass
import concourse.tile as tile
import concourse.mybir as mybir
from concourse.bass_utils import run_bass_kernel_spmd

# Pin ACT table-set choice: the greedy first-match in insert_act_table_loads
# would alternate natural_log <-> exp_and_others every DAG step (2 x 2.7us per
# step). Advertise ln/exp only in natural_log_exp_and_others and tanh/sigmoid
# only in sigmoid_and_others so the whole kernel needs exactly 2 table loads.
_ORIG_GAT = bacc.get_activation_tables


def _pinned_activation_tables(arch):
    tables = _ORIG_GAT(arch)
    LN = mybir.ActivationFunctionType.Ln
    EXP = mybir.ActivationFunctionType.Exp
    TANH = mybir.ActivationFunctionType.Tanh
    SIG = mybir.ActivationFunctionType.Sigmoid
    for name, funcs in tables.items():
        if name != "natural_log_exp_and_others":
            funcs.discard(LN)
            funcs.discard(EXP)
        if name != "sigmoid_and_others":
            funcs.discard(TANH)
            funcs.discard(SIG)
    return tables


bacc.get_activation_tables = _pinned_activation_tables

F32 = mybir.dt.float32
F16 = mybir.dt.float16
U32 = mybir.dt.uint32
ALU = mybir.AluOpType
ACTF = mybir.ActivationFunctionType
AXX = mybir.AxisListType.X

NCORES = 8
B, T, H = 4, 4096, 2048
NTOK = B * T                    # 16384
TPC = NTOK // NCORES            # 2048 tokens per core
NTILE = TPC // 128              # 16 token tiles per core
KCH = H // 128                  # 16 contraction chunks
NN = 16                         # DAG nodes
INTER = 8                       # steps
INIT_SLOTS = 8
NF = 168                        # 32 init + 128 op + 8 gate
LOG_CLAMP = 23.026
SCL = 2048.0                    # 2^11 lo-part scale
ISCL = 1.0 / SCL
NCHUNKS = 2                     # DAG token-chunks (overlap DAG with later matmuls)
BATCH_EXP = True                # batch et+er exps into one ACT call per step
SPLIT_R = True                  # prefix/last-col split of the R reductions

_CACHE = {}


def _build(repeats=1, parts="all"):
    nc = bacc.Bacc("TRN2", target_bir_lowering=False, debug=False)

    hf_d = nc.dram_tensor("hf", [NTILE, H, 256], F16, kind="ExternalInput")
    wt_d = nc.dram_tensor("wt", [H, 2 * NF], F16, kind="ExternalInput")
    bias_d = nc.dram_tensor("bias", [1, 2 * NF], F16, kind="ExternalInput")
    out_d = nc.dram_tensor("out", [128, NTILE], F32, kind="ExternalOutput")

    with tile.TileContext(nc) as tc:
        with tc.tile_pool(name="consts", bufs=1) as consts, \
             tc.tile_pool(name="hfp", bufs=3) as hfp, \
             tc.tile_pool(name="evp", bufs=3) as evp, \
             tc.tile_pool(name="ns", bufs=2) as ns, \
             tc.tile_pool(name="pp", bufs=3, space="PSUM") as pp:

            wt_sb = consts.tile([128, KCH, 2 * NF], F16)
            nc.sync.dma_start(out=wt_sb, in_=wt_d.rearrange("(k p) f -> p k f", p=128))
            bias_sb = consts.tile([1, 2 * NF], F16)
            nc.sync.dma_start(out=bias_sb, in_=bias_d[:, :])
            ones = consts.tile([1, 128], F16)
            nc.vector.memset(ones, 1.0)

            for _rep in range(repeats):
                _emit_body(nc, tc, consts, hfp, evp, ns, pp,
                           hf_d, wt_sb, bias_sb, ones, out_d, parts=parts)

    nc.compile()
    return nc


def _emit_body(nc, tc, consts, hfp, evp, ns, pp, hf_d, wt_sb, bias_sb, ones, out_d,
               parts="all"):
    cw = NTILE // NCHUNKS
    chunks = [(c * cw, (c + 1) * cw, chr(65 + c)) for c in range(NCHUNKS)]
    plan = {sfx: consts.tile([128, cw, NF], F32, tag=f"plan{sfx}", name=f"plan{sfx}")
            for _, _, sfx in chunks}

    # ---- plan predictor: 3-pass fp16 matmul per token tile ----
    if parts == "dag":
        for _, _, sfx in chunks:
            nc.vector.memset(plan[sfx], 0.5)
    for i in range(NTILE) if parts != "dag" else []:
        lo_t, _, sfx = chunks[i // cw]
        hf_sb = hfp.tile([128, KCH, 256], F16, tag="hf")
        nc.sync.dma_start(
            out=hf_sb, in_=hf_d[i].rearrange("(k p) ct -> p k ct", p=128)
        )
        p12 = pp.tile([128, 2 * NF], F32, tag="p12")
        nc.tensor.matmul(p12, ones[:, :], bias_sb[:, :], start=True, stop=False)
        for k in range(KCH):
            hi = hf_sb[:, k, 0:128]
            lo = hf_sb[:, k, 128:256]
            wh = wt_sb[:, k, 0:NF]
            whl = wt_sb[:, k, :]
            # hi * [Wh | Wl] in one N=336 matmul; lo * Wh into the hi-Wl half
            nc.tensor.matmul(p12, hi, whl, start=False, stop=False)
            nc.tensor.matmul(p12[:, NF:2 * NF], lo, wh, start=False,
                             stop=(k == KCH - 1), skip_group_check=True)
        tmp = evp.tile([128, NF], F32, tag="ev")
        nc.scalar.activation(tmp, p12[:, NF:2 * NF], ACTF.Copy, bias=0.0, scale=ISCL)
        nc.vector.tensor_tensor(out=plan[sfx][:, i - lo_t, :], in0=p12[:, 0:NF],
                                in1=tmp, op=ALU.add)

    if parts == "mm":
        nc.sync.dma_start(out=out_d[:, :], in_=plan[chunks[0][2]][:, 0, 0:NTILE])
        return

    st = {}
    for _, _, sfx in chunks:
        st[sfx] = _dag_init(nc, consts, ns, plan[sfx], sfx, cw)
    for s in range(INTER):
        for _, _, sfx in chunks:
            _dag_step(nc, ns, plan[sfx], st[sfx], sfx, cw, s)
    for lo_t, hi_t, sfx in chunks:
        nc.sync.dma_start(out=out_d[:, lo_t:hi_t], in_=st[sfx]["OUT"])


def _dag_init(nc, consts, ns, PLAN, sfx, cw):
    st = {}
    G = st["G"] = consts.tile([128, cw, INTER], F32, tag=f"G{sfx}", name=f"G{sfx}")
    VSIGN = st["VSIGN"] = consts.tile([128, cw, NN], F32, tag=f"VSIGN{sfx}", name=f"VSIGN{sfx}")
    VMAG = consts.tile([128, cw, NN], F32, tag=f"VMAG{sfx}")
    LMD = st["LMD"] = consts.tile([128, 2, cw, NN], F32, tag=f"LMD{sfx}", name=f"LMD{sfx}")
    LOGMAG = LMD[:, 0, :, :]
    DIFF = LMD[:, 1, :, :]
    AONE = st["AONE"] = consts.tile([128, cw, NN], F32, tag=f"AONE{sfx}", name=f"AONE{sfx}")
    PVT = st["PVT"] = consts.tile([128, cw, INTER], F32, tag=f"PVT{sfx}", name=f"PVT{sfx}")
    st["OUT"] = consts.tile([128, cw], F32, tag=f"OUT{sfx}", name=f"OUT{sfx}")
    nc.vector.memset(AONE, 1.0)

    # sigmoid_and_others table set: Tanh + Sigmoid
    nc.scalar.activation(VSIGN, PLAN[:, :, 16:32], ACTF.Tanh)
    nc.scalar.activation(G, PLAN[:, :, 160:168], ACTF.Sigmoid)
    vabs = ns.tile([128, cw, NN], F32, tag=f"vabs{sfx}")
    nc.vector.tensor_scalar(out=vabs.bitcast(U32), in0=PLAN[:, :, 0:16].bitcast(U32),
                            scalar1=0x7FFFFFFF, scalar2=None, op0=ALU.bitwise_and)
    nc.vector.tensor_scalar(out=VMAG, in0=vabs, scalar1=1e-12, scalar2=None, op0=ALU.max)
    # natural_log_exp_and_others table set from here on
    nc.scalar.activation(LOGMAG, VMAG, ACTF.Ln)
    sg0 = ns.tile([128, cw, NN], F32, tag=f"sg0{sfx}")
    nc.vector.tensor_tensor(out=sg0, in0=VSIGN, in1=VMAG, op=ALU.mult)
    nc.vector.tensor_tensor(out=DIFF, in0=sg0, in1=LOGMAG, op=ALU.subtract)

    # PV head init: prod of V_sign slots 0..7
    pva = ns.tile([128, cw, 4], F32, tag=f"pva{sfx}")
    nc.vector.tensor_tensor(out=pva, in0=VSIGN[:, :, 0:4], in1=VSIGN[:, :, 4:8], op=ALU.mult)
    pvb = ns.tile([128, cw, 2], F32, tag=f"pvb{sfx}")
    nc.vector.tensor_tensor(out=pvb, in0=pva[:, :, 0:2], in1=pva[:, :, 2:4], op=ALU.mult)
    pv = ns.tile([128, cw], F32, tag=f"pv{sfx}")
    nc.vector.tensor_tensor(out=pv, in0=pvb[:, :, 0], in1=pvb[:, :, 1], op=ALU.mult)
    st["pv"] = pv

    # PV tail suffix products: PVT[:, :, s] = prod_{j >= 8+s} V_sign_init[j]
    nc.vector.tensor_copy(out=PVT[:, :, INTER - 1], in_=VSIGN[:, :, NN - 1])
    for j in range(INTER - 2, -1, -1):
        nc.vector.tensor_tensor(out=PVT[:, :, j], in0=PVT[:, :, j + 1],
                                in1=VSIGN[:, :, 8 + j], op=ALU.mult)
    return st


def _dag_step(nc, ns, PLAN, st, sfx, cw, s):
    G, LMD, AONE, PVT = st["G"], st["LMD"], st["AONE"], st["PVT"]
    LOGMAG = LMD[:, 0, :, :]
    DIFF = LMD[:, 1, :, :]
    v = INIT_SLOTS + s          # valid node count (mask: pos < v)
    O_s = PLAN[:, :, 32 + 16 * s: 32 + 16 * s + v]
    G_s = G[:, :, s]

    def T(nm, shape=None, dt=F32):
        return ns.tile(shape or [128, cw], dt, tag=f"{nm}{sfx}", name=f"{nm}{sfx}")

    vp = v - 1 if (SPLIT_R and s > 0) else v
    m12 = T("m12", [128, cw, 2, NN])
    ob = bass.AP(tensor=O_s.tensor, offset=O_s.offset,
                 ap=[O_s.ap[0], O_s.ap[1], [0, 2], [O_s.ap[2][0], vp]])
    lv = bass.AP(tensor=LMD.tensor, offset=LMD.offset + 0,
                 ap=[LMD.ap[0], LMD.ap[2], LMD.ap[1], [LMD.ap[3][0], vp]])
    nc.vector.tensor_tensor(out=m12[:, :, :, :vp], in0=ob, in1=lv, op=ALU.mult)
    r12 = T("r12", [128, cw, 2])
    nc.vector.tensor_reduce(out=r12, in_=m12[:, :, :, :vp], op=ALU.add, axis=AXX)
    if vp != v:
        # add the newest column's contribution (prefix computed without it so the
        # previous step's Ln/diff stays off this step's launch dependencies)
        nnode = v - 1
        oc = bass.AP(tensor=O_s.tensor, offset=O_s.offset + nnode,
                     ap=[O_s.ap[0], O_s.ap[1], [0, 2]])
        lc = bass.AP(tensor=LMD.tensor, offset=LMD.offset + nnode,
                     ap=[LMD.ap[0], LMD.ap[2], LMD.ap[1]])
        tnewc = T("tnewc", [128, cw, 2])
        nc.vector.tensor_tensor(out=tnewc, in0=oc, in1=lc, op=ALU.mult)
        r12f = T("r12f", [128, cw, 2])
        nc.vector.tensor_tensor(out=r12f, in0=r12, in1=tnewc, op=ALU.add)
        r12 = r12f
    r1 = r12[:, :, 0]
    r2 = r12[:, :, 1]

    tmp32 = T("tmp32", [128, 2, cw])
    R = tmp32[:, 0, :]
    SP = tmp32[:, 1, :]
    gr2 = T("gr2")
    nc.vector.tensor_tensor(out=gr2, in0=G_s, in1=r2, op=ALU.mult)
    nc.vector.tensor_tensor(out=R, in0=r1, in1=gr2, op=ALU.add)

    # sign_prod = pv * PVT[s] * prod_{j<v}(|O_s|+1)
    oabs = T("oabs", [128, cw, NN])
    nc.vector.tensor_scalar(out=oabs[:, :, :v].bitcast(U32), in0=O_s.bitcast(U32),
                            scalar1=0x7FFFFFFF, scalar2=None, op0=ALU.bitwise_and)
    nc.vector.tensor_scalar(out=AONE[:, :, :v], in0=oabs[:, :, :v],
                            scalar1=1.0, scalar2=None, op0=ALU.add)
    t8 = T("t8", [128, cw, 8])
    nc.vector.tensor_tensor(out=t8, in0=AONE[:, :, 0:8], in1=AONE[:, :, 8:16], op=ALU.mult)
    t4 = T("t4", [128, cw, 4])
    nc.vector.tensor_tensor(out=t4, in0=t8[:, :, 0:4], in1=t8[:, :, 4:8], op=ALU.mult)
    t2 = T("t2", [128, cw, 2])
    nc.vector.tensor_tensor(out=t2, in0=t4[:, :, 0:2], in1=t4[:, :, 2:4], op=ALU.mult)
    t1 = T("t1")
    nc.vector.tensor_tensor(out=t1, in0=t2[:, :, 0], in1=t2[:, :, 1], op=ALU.mult)
    pvc = T("pvc")
    nc.vector.tensor_tensor(out=pvc, in0=st["pv"], in1=PVT[:, :, s], op=ALU.mult)
    nc.vector.tensor_tensor(out=SP, in0=t1, in1=pvc, op=ALU.mult)

    # lin_sign/log_sign = tanh(tmp32 * 1e4) via exp, batched [128, 2*cw]
    if BATCH_EXP:
        ax = T("ax", [128, 2, cw])
        nc.vector.tensor_scalar(out=ax.bitcast(U32), in0=tmp32.bitcast(U32),
                                scalar1=0x7FFFFFFF, scalar2=None, op0=ALU.bitwise_and)
        expin = T("expin", [128, 3, cw])
        nc.vector.tensor_scalar(out=expin[:, 0:2, :], in0=ax, scalar1=-2.0e4,
                                scalar2=None, op0=ALU.mult)
        nc.vector.tensor_scalar(out=expin[:, 2, :], in0=tmp32[:, 0, :],
                                scalar1=LOG_CLAMP, scalar2=None, op0=ALU.min)
        eo = T("eo", [128, 3, cw])
        nc.scalar.activation(eo, expin, ACTF.Exp)
        et = eo[:, 0:2, :]
    else:
        ax = T("ax", [128, 2, cw])
        nc.vector.tensor_scalar(out=ax.bitcast(U32), in0=tmp32.bitcast(U32),
                                scalar1=0x7FFFFFFF, scalar2=None, op0=ALU.bitwise_and)
        et = T("et", [128, 2, cw])
        nc.scalar.activation(et, ax, ACTF.Exp, bias=0.0, scale=-2.0e4)
    num = T("num", [128, 2, cw])
    nc.vector.tensor_scalar(out=num, in0=et, scalar1=-1.0, scalar2=1.0, op0=ALU.mult, op1=ALU.add)
    den = T("den", [128, 2, cw])
    nc.vector.tensor_scalar(out=den, in0=et, scalar1=1.0, scalar2=None, op0=ALU.add)
    rd = T("rd", [128, 2, cw])
    nc.vector.reciprocal(out=rd, in_=den)
    uu = T("uu", [128, 2, cw])
    nc.vector.tensor_tensor(out=uu, in0=num, in1=rd, op=ALU.mult)
    sgn = T("sgn", [128, 2, cw], U32)
    nc.vector.tensor_scalar(out=sgn, in0=tmp32.bitcast(U32),
                            scalar1=0x80000000, scalar2=None, op0=ALU.bitwise_and)
    tnh = T("tnh", [128, 2, cw])
    nc.vector.tensor_tensor(out=tnh.bitcast(U32), in0=uu.bitcast(U32), in1=sgn, op=ALU.bitwise_xor)
    lin = tnh[:, 0, :]
    lgs = tnh[:, 1, :]

    # Vm = exp(min(R, clamp)) + G*(|R| - exp(...))
    if BATCH_EXP:
        er = eo[:, 2, :]
    else:
        minr = T("minr")
        nc.vector.tensor_scalar(out=minr, in0=R, scalar1=LOG_CLAMP, scalar2=None, op0=ALU.min)
        er = T("er")
        nc.scalar.activation(er, minr, ACTF.Exp)
    ar = T("ar")
    nc.vector.tensor_scalar(out=ar.bitcast(U32), in0=R.bitcast(U32),
                            scalar1=0x7FFFFFFF, scalar2=None, op0=ALU.bitwise_and)
    d1 = T("d1")
    nc.vector.tensor_tensor(out=d1, in0=ar, in1=er, op=ALU.subtract)
    d2 = T("d2")
    nc.vector.tensor_tensor(out=d2, in0=G_s, in1=d1, op=ALU.mult)
    vm = T("vm")
    nc.vector.tensor_tensor(out=vm, in0=er, in1=d2, op=ALU.add)

    # Vs = log_sign + G*(lin_sign - log_sign)
    e1 = T("e1")
    nc.vector.tensor_tensor(out=e1, in0=lin, in1=lgs, op=ALU.subtract)
    e2 = T("e2")
    nc.vector.tensor_tensor(out=e2, in0=G_s, in1=e1, op=ALU.mult)
    vs = T("vs")
    nc.vector.tensor_tensor(out=vs, in0=lgs, in1=e2, op=ALU.add)

    if s == INTER - 1:
        nc.vector.tensor_tensor(out=st["OUT"], in0=vs, in1=vm, op=ALU.mult)
    else:
        idx = INIT_SLOTS + s
        sgnew = T("sgnew")
        nc.vector.tensor_tensor(out=sgnew, in0=vs, in1=vm, op=ALU.mult)
        vmc = T("vmc")
        nc.vector.tensor_scalar(out=vmc, in0=vm, scalar1=1e-12, scalar2=None, op0=ALU.max)
        nc.scalar.activation(LOGMAG[:, :, idx], vmc, ACTF.Ln)
        nc.vector.tensor_tensor(out=DIFF[:, :, idx], in0=sgnew,
                                in1=LOGMAG[:, :, idx], op=ALU.subtract)
        pv_next = ns.tile([128, cw], F32, tag=f"pv{sfx}")
        nc.vector.tensor_tensor(out=pv_next, in0=st["pv"], in1=vs, op=ALU.mult)
        st["pv"] = pv_next


def _get_nc():
    if "nc" not in _CACHE:
        _CACHE["nc"] = _build()
    return _CACHE["nc"]


def _prep_inputs(hidden, W_init, b_init, W_op, b_op, W_gate, b_gate):
    hidden = np.ascontiguousarray(np.asarray(hidden, np.float32)).reshape(NTOK, H)
    Wcat = np.concatenate([np.asarray(W_init, np.float32),
                           np.asarray(W_op, np.float32),
                           np.asarray(W_gate, np.float32)], axis=0)   # [168, H]
    bcat = np.concatenate([np.asarray(b_init, np.float32),
                           np.asarray(b_op, np.float32),
                           np.asarray(b_gate, np.float32)])           # [168]

    WT = np.ascontiguousarray(Wcat.T)                                  # [H, 168]
    Wh = WT.astype(np.float16)
    Wl = ((WT - Wh.astype(np.float32)) * SCL).astype(np.float16)
    wt = np.concatenate([Wh, Wl], axis=1)                              # [H, 336]

    bh = bcat.astype(np.float16)
    bl = ((bcat - bh.astype(np.float32)) * SCL).astype(np.float16)
    bias = np.concatenate([bh, bl])[None, :]                           # [1, 336]

    in_maps = []
    for c in range(NCORES):
        shard = hidden[c * TPC:(c + 1) * TPC]                          # [2048, H]
        hT = np.ascontiguousarray(shard.T)                             # [H, 2048]
        fh = hT.astype(np.float16)
        fl = ((hT - fh.astype(np.float32)) * SCL).astype(np.float16)
        # [NTILE, H, 256]: per tile i, [h, 0:128] = hi of tokens, [h, 128:256] = lo
        comb = np.empty((NTILE, H, 256), np.float16)
        for i in range(NTILE):
            comb[i, :, 0:128] = fh[:, i * 128:(i + 1) * 128]
            comb[i, :, 128:256] = fl[:, i * 128:(i + 1) * 128]
        in_maps.append({"hf": comb, "wt": wt, "bias": bias})
    return in_maps


def _run(in_maps, **kwargs):
    nc = _get_nc()
    return run_bass_kernel_spmd(nc, in_maps, core_ids=list(range(NCORES)), **kwargs)


def _assemble(results):
    out = np.empty((NTOK,), np.float32)
    for c in range(NCORES):
        out[c * TPC:(c + 1) * TPC] = results[c]["out"].T.reshape(TPC)
    return out.reshape(B, T)


def kernel(**inputs):
    in_maps = _prep_inputs(**inputs)
    res = _run(in_maps)
    return _assemble(res.results)


def kernel_traced(**inputs):
    """Like kernel() but with NTFF tracing; returns (output, BassKernelResults)."""
    in_maps = _prep_inputs(**inputs)
    res = _run(in_maps, trace=True)
    return _assemble(res.results), res



# revision 2
# speedup vs baseline: 440.3221x; 440.3221x over previous
"""Trainium2 Bass kernel for nn_NewDAGExecutor (plan-predictor matmul + 8-step DAG).

Strategy (8 NeuronCores, data-parallel over the 16384 tokens, 2048 tokens/core):
  - Host: transpose each core's token shard to [H, tok], split fp32 into an
    exact fp16 hi/lo pair (lo scaled by 2^11), same for the concatenated
    weight matrix W = [W_init; W_op; W_gate] (168 outputs) and biases.
  - Device: plan = hidden @ W.T + b computed as 3 fp16 matmul passes
    (hi*Whi -> PSUM1, hi*Wlo + lo*Whi -> PSUM2, combine P1 + 2^-11*P2),
    which is fp32-accurate (fp16 products are exact in the fp32 MACs; the
    dropped lo*lo term is ~2^-24 relative). Tokens ride the PSUM partition
    axis so the DAG math lands in token-major layout with no transpose.
  - The 8 sequential DAG steps run wide across all 2048 tokens/core
    ([128 partitions x 16 tile-columns]) on DVE/ACT. tanh(x*1e4) is computed
    via exp to keep every per-step ACT call inside the natural_log_exp table
    set (no per-step table switches); the sign is restored with bitwise ops.
"""

import numpy as np

import concourse.bacc as bacc
import concourse.bass as bass
import concourse.tile as tile
import concourse.mybir as mybir
from concourse.bass_utils import run_bass_kernel_spmd

# Pin ACT table-set choice: the greedy first-match in insert_act_table_loads
# would alternate natural_log <-> exp_and_others every DAG step (2 x 2.7us per
# step). Advertise ln/exp only in natural_log_exp_and_others and tanh/sigmoid
# only in sigmoid_and_others so the whole kernel needs exactly 2 table loads.
_ORIG_GAT = bacc.get_activation_tables


def _pinned_activation_tables(arch):
    tables = _ORIG_GAT(arch)
    LN = mybir.ActivationFunctionType.Ln
    EXP = mybir.ActivationFunctionType.Exp
    TANH = mybir.ActivationFunctionType.Tanh
    SIG = mybir.ActivationFunctionType.Sigmoid
    for name, funcs in tables.items():
        if name != "natural_log_exp_and_others":
            funcs.discard(LN)
            funcs.discard(EXP)
        if name != "sigmoid_and_others":
            funcs.discard(TANH)
            funcs.discard(SIG)
    return tables


bacc.get_activation_tables = _pinned_activation_tables

F32 = mybir.dt.float32
F16 = mybir.dt.float16
U32 = mybir.dt.uint32
ALU = mybir.AluOpType
ACTF = mybir.ActivationFunctionType
AXX = mybir.AxisListType.X

NCORES = 8
B, T, H = 4, 4096, 2048
NTOK = B * T                    # 16384
TPC = NTOK // NCORES            # 2048 tokens per core
NTILE = TPC // 128              # 16 token tiles per core
KCH = H // 128                  # 16 contraction chunks
NN = 16                         # DAG nodes
INTER = 8                       # steps
INIT_SLOTS = 8
NF = 168                        # 32 init + 128 op + 8 gate
LOG_CLAMP = 23.026
SCL = 2048.0                    # 2^11 lo-part scale
ISCL = 1.0 / SCL
NCHUNKS = 2                     # DAG token-chunks (overlap DAG with later matmuls)
BATCH_EXP = True                # batch et+er exps into one ACT call per step
SPLIT_R = True                  # prefix/last-col split of the R reductions

_CACHE = {}


def _build(repeats=1, parts="all"):
    nc = bacc.Bacc("TRN2", target_bir_lowering=False, debug=False)

    hf_d = nc.dram_tensor("hf", [NTILE, H, 256], F16, kind="ExternalInput")
    wt_d = nc.dram_tensor("wt", [H, 2 * NF], F16, kind="ExternalInput")
    bias_d = nc.dram_tensor("bias", [1, 2 * NF], F16, kind="ExternalInput")
    out_d = nc.dram_tensor("out", [128, NTILE], F32, kind="ExternalOutput")

    with tile.TileContext(nc) as tc:
        with tc.tile_pool(name="consts", bufs=1) as consts, \
             tc.tile_pool(name="hfp", bufs=3) as hfp, \
             tc.tile_pool(name="evp", bufs=3) as evp, \
             tc.tile_pool(name="ns", bufs=2) as ns, \
             tc.tile_pool(name="pp", bufs=3, space="PSUM") as pp:

            wt_sb = consts.tile([128, KCH, 2 * NF], F16)
            nc.sync.dma_start(out=wt_sb, in_=wt_d.rearrange("(k p) f -> p k f", p=128))
            bias_sb = consts.tile([1, 2 * NF], F16)
            nc.sync.dma_start(out=bias_sb, in_=bias_d[:, :])
            ones = consts.tile([1, 128], F16)
            nc.vector.memset(ones, 1.0)

            for _rep in range(repeats):
                _emit_body(nc, tc, consts, hfp, evp, ns, pp,
                           hf_d, wt_sb, bias_sb, ones, out_d, parts=parts)

    nc.compile()
    return nc


def _emit_body(nc, tc, consts, hfp, evp, ns, pp, hf_d, wt_sb, bias_sb, ones, out_d,
               parts="all"):
    cw = NTILE // NCHUNKS
    chunks = [(c * cw, (c + 1) * cw, chr(65 + c)) for c in range(NCHUNKS)]
    plan = {sfx: consts.tile([128, cw, NF], F32, tag=f"plan{sfx}", name=f"plan{sfx}")
            for _, _, sfx in chunks}

    # ---- plan predictor: 3-pass fp16 matmul per token tile ----
    if parts == "dag":
        for _, _, sfx in chunks:
            nc.vector.memset(plan[sfx], 0.5)
    for i in range(NTILE) if parts != "dag" else []:
        lo_t, _, sfx = chunks[i // cw]
        hf_sb = hfp.tile([128, KCH, 256], F16, tag="hf")
        nc.sync.dma_start(
            out=hf_sb, in_=hf_d[i].rearrange("(k p) ct -> p k ct", p=128)
        )
        p12 = pp.tile([128, 2 * NF], F32, tag="p12")
        nc.tensor.matmul(p12, ones[:, :], bias_sb[:, :], start=True, stop=False)
        for k in range(KCH):
            hi = hf_sb[:, k, 0:128]
            lo = hf_sb[:, k, 128:256]
            wh = wt_sb[:, k, 0:NF]
            whl = wt_sb[:, k, :]
            # hi * [Wh | Wl] in one N=336 matmul; lo * Wh into the hi-Wl half
            nc.tensor.matmul(p12, hi, whl, start=False, stop=False)
            nc.tensor.matmul(p12[:, NF:2 * NF], lo, wh, start=False,
                             stop=(k == KCH - 1), skip_group_check=True)
        tmp = evp.tile([128, NF], F32, tag="ev")
        nc.scalar.activation(tmp, p12[:, NF:2 * NF], ACTF.Copy, bias=0.0, scale=ISCL)
        nc.vector.tensor_tensor(out=plan[sfx][:, i - lo_t, :], in0=p12[:, 0:NF],
                                in1=tmp, op=ALU.add)

    if parts == "mm":
        nc.sync.dma_start(out=out_d[:, :], in_=plan[chunks[0][2]][:, 0, 0:NTILE])
        return

    st = {}
    for _, _, sfx in chunks:
        st[sfx] = _dag_init(nc, consts, ns, plan[sfx], sfx, cw)
    for s in range(INTER):
        for _, _, sfx in chunks:
            _dag_step(nc, ns, plan[sfx], st[sfx], sfx, cw, s)
    for lo_t, hi_t, sfx in chunks:
        nc.sync.dma_start(out=out_d[:, lo_t:hi_t], in_=st[sfx]["OUT"])


def _dag_init(nc, consts, ns, PLAN, sfx, cw):
    st = {}
    G = st["G"] = consts.tile([128, cw, INTER], F32, tag=f"G{sfx}", name=f"G{sfx}")
    VSIGN = st["VSIGN"] = consts.tile([128, cw, NN], F32, tag=f"VSIGN{sfx}", name=f"VSIGN{sfx}")
    VMAG = consts.tile([128, cw, NN], F32, tag=f"VMAG{sfx}")
    LMD = st["LMD"] = consts.tile([128, 2, cw, NN], F32, tag=f"LMD{sfx}", name=f"LMD{sfx}")
    LOGMAG = LMD[:, 0, :, :]
    DIFF = LMD[:, 1, :, :]
    AONE = st["AONE"] = consts.tile([128, cw, NN], F32, tag=f"AONE{sfx}", name=f"AONE{sfx}")
    PVT = st["PVT"] = consts.tile([128, cw, INTER], F32, tag=f"PVT{sfx}", name=f"PVT{sfx}")
    st["OUT"] = consts.tile([128, cw], F32, tag=f"OUT{sfx}", name=f"OUT{sfx}")
    nc.vector.memset(AONE, 1.0)

    # sigmoid_and_others table set: Tanh + Sigmoid
    nc.scalar.activation(VSIGN, PLAN[:, :, 16:32], ACTF.Tanh)
    nc.scalar.activation(G, PLAN[:, :, 160:168], ACTF.Sigmoid)
    vabs = ns.tile([128, cw, NN], F32, tag=f"vabs{sfx}")
    nc.vector.tensor_scalar(out=vabs.bitcast(U32), in0=PLAN[:, :, 0:16].bitcast(U32),
                            scalar1=0x7FFFFFFF, scalar2=None, op0=ALU.bitwise_and)
    nc.vector.tensor_scalar(out=VMAG, in0=vabs, scalar1=1e-12, scalar2=None, op0=ALU.max)
    # natural_log_exp_and_others table set from here on
    nc.scalar.activation(LOGMAG, VMAG, ACTF.Ln)
    sg0 = ns.tile([128, cw, NN], F32, tag=f"sg0{sfx}")
    nc.vector.tensor_tensor(out=sg0, in0=VSIGN, in1=VMAG, op=ALU.mult)
    nc.vector.tensor_tensor(out=DIFF, in0=sg0, in1=LOGMAG, op=ALU.subtract)

    # PV head init: prod of V_sign slots 0..7
    pva = ns.tile([128, cw, 4], F32, tag=f"pva{sfx}")
    nc.vector.tensor_tensor(out=pva, in0=VSIGN[:, :, 0:4], in1=VSIGN[:, :, 4:8], op=ALU.mult)
    pvb = ns.tile([128, cw, 2], F32, tag=f"pvb{sfx}")
    nc.vector.tensor_tensor(out=pvb, in0=pva[:, :, 0:2], in1=pva[:, :, 2:4], op=ALU.mult)
    pv = ns.tile([128, cw], F32, tag=f"pv{sfx}")
    nc.vector.tensor_tensor(out=pv, in0=pvb[:, :, 0], in1=pvb[:, :, 1], op=ALU.mult)
    st["pv"] = pv

    # PV tail suffix products: PVT[:, :, s] = prod_{j >= 8+s} V_sign_init[j]
    nc.vector.tensor_copy(out=PVT[:, :, INTER - 1], in_=VSIGN[:, :, NN - 1])
    for j in range(INTER - 2, -1, -1):
        nc.vector.tensor_tensor(out=PVT[:, :, j], in0=PVT[:, :, j + 1],
                                in1=VSIGN[:, :, 8 + j], op=ALU.mult)
    return st


def _dag_step(nc, ns, PLAN, st, sfx, cw, s):
    G, LMD, AONE, PVT = st["G"], st["LMD"], st["AONE"], st["PVT"]
    LOGMAG = LMD[:, 0, :, :]
    DIFF = LMD[:, 1, :, :]
    v = INIT_SLOTS + s          # valid node count (mask: pos < v)
    O_s = PLAN[:, :, 32 + 16 * s: 32 + 16 * s + v]
    G_s = G[:, :, s]

    def T(nm, shape=None, dt=F32):
        return ns.tile(shape or [128, cw], dt, tag=f"{nm}{sfx}", name=f"{nm}{sfx}")

    vp = v - 1 if (SPLIT_R and s > 0) else v
    m12 = T("m12", [128, cw, 2, NN])
    ob = bass.AP(tensor=O_s.tensor, offset=O_s.offset,
                 ap=[O_s.ap[0], O_s.ap[1], [0, 2], [O_s.ap[2][0], vp]])
    lv = bass.AP(tensor=LMD.tensor, offset=LMD.offset + 0,
                 ap=[LMD.ap[0], LMD.ap[2], LMD.ap[1], [LMD.ap[3][0], vp]])
    nc.vector.tensor_tensor(out=m12[:, :, :, :vp], in0=ob, in1=lv, op=ALU.mult)
    r12 = T("r12", [128, cw, 2])
    nc.vector.tensor_reduce(out=r12, in_=m12[:, :, :, :vp], op=ALU.add, axis=AXX)
    if vp != v:
        # add the newest column's contribution (prefix computed without it so the
        # previous step's Ln/diff stays off this step's launch dependencies)
        nnode = v - 1
        oc = bass.AP(tensor=O_s.tensor, offset=O_s.offset + nnode,
                     ap=[O_s.ap[0], O_s.ap[1], [0, 2]])
        lc = bass.AP(tensor=LMD.tensor, offset=LMD.offset + nnode,
                     ap=[LMD.ap[0], LMD.ap[2], LMD.ap[1]])
        tnewc = T("tnewc", [128, cw, 2])
        nc.vector.tensor_tensor(out=tnewc, in0=oc, in1=lc, op=ALU.mult)
        r12f = T("r12f", [128, cw, 2])
        nc.vector.tensor_tensor(out=r12f, in0=r12, in1=tnewc, op=ALU.add)
        r12 = r12f
    r1 = r12[:, :, 0]
    r2 = r12[:, :, 1]

    tmp32 = T("tmp32", [128, 2, cw])
    R = tmp32[:, 0, :]
    SP = tmp32[:, 1, :]
    gr2 = T("gr2")
    nc.vector.tensor_tensor(out=gr2, in0=G_s, in1=r2, op=ALU.mult)
    nc.vector.tensor_tensor(out=R, in0=r1, in1=gr2, op=ALU.add)

    # sign_prod = pv * PVT[s] * prod_{j<v}(|O_s|+1)
    oabs = T("oabs", [128, cw, NN])
    nc.vector.tensor_scalar(out=oabs[:, :, :v].bitcast(U32), in0=O_s.bitcast(U32),
                            scalar1=0x7FFFFFFF, scalar2=None, op0=ALU.bitwise_and)
    nc.vector.tensor_scalar(out=AONE[:, :, :v], in0=oabs[:, :, :v],
                            scalar1=1.0, scalar2=None, op0=ALU.add)
    t8 = T("t8", [128, cw, 8])
    nc.vector.tensor_tensor(out=t8, in0=AONE[:, :, 0:8], in1=AONE[:, :, 8:16], op=ALU.mult)
    t4 = T("t4", [128, cw, 4])
    nc.vector.tensor_tensor(out=t4, in0=t8[:, :, 0:4], in1=t8[:, :, 4:8], op=ALU.mult)
    t2 = T("t2", [128, cw, 2])
    nc.vector.tensor_tensor(out=t2, in0=t4[:, :, 0:2], in1=t4[:, :, 2:4], op=ALU.mult)
    t1 = T("t1")
    nc.vector.tensor_tensor(out=t1, in0=t2[:, :, 0], in1=t2[:, :, 1], op=ALU.mult)
    pvc = T("pvc")
    nc.vector.tensor_tensor(out=pvc, in0=st["pv"], in1=PVT[:, :, s], op=ALU.mult)
    nc.vector.tensor_tensor(out=SP, in0=t1, in1=pvc, op=ALU.mult)

    # lin_sign/log_sign = tanh(tmp32 * 1e4) via exp, batched [128, 2*cw]
    if BATCH_EXP:
        ax = T("ax", [128, 2, cw])
        nc.vector.tensor_scalar(out=ax.bitcast(U32), in0=tmp32.bitcast(U32),
                                scalar1=0x7FFFFFFF, scalar2=None, op0=ALU.bitwise_and)
        expin = T("expin", [128, 3, cw])
        nc.vector.tensor_scalar(out=expin[:, 0:2, :], in0=ax, scalar1=-2.0e4,
                                scalar2=None, op0=ALU.mult)
        nc.vector.tensor_scalar(out=expin[:, 2, :], in0=tmp32[:, 0, :],
                                scalar1=LOG_CLAMP, scalar2=None, op0=ALU.min)
        eo = T("eo", [128, 3, cw])
        nc.scalar.activation(eo, expin, ACTF.Exp)
        et = eo[:, 0:2, :]
    else:
        ax = T("ax", [128, 2, cw])
        nc.vector.tensor_scalar(out=ax.bitcast(U32), in0=tmp32.bitcast(U32),
                                scalar1=0x7FFFFFFF, scalar2=None, op0=ALU.bitwise_and)
        et = T("et", [128, 2, cw])
        nc.scalar.activation(et, ax, ACTF.Exp, bias=0.0, scale=-2.0e4)
    num = T("num", [128, 2, cw])
    nc.vector.tensor_scalar(out=num, in0=et, scalar1=-1.0, scalar2=1.0, op0=ALU.mult, op1=ALU.add)
    den = T("den", [128, 2, cw])
    nc.vector.tensor_scalar(out=den, in0=et, scalar1=1.0, scalar2=None, op0=ALU.add)
    rd = T("rd", [128, 2, cw])
    nc.vector.reciprocal(out=rd, in_=den)
    uu = T("uu", [128, 2, cw])
    nc.vector.tensor_tensor(out=uu, in0=num, in1=rd, op=ALU.mult)
    sgn = T("sgn", [128, 2, cw], U32)
    nc.vector.tensor_scalar(out=sgn, in0=tmp32.bitcast(U32),
                            scalar1=0x80000000, scalar2=None, op0=ALU.bitwise_and)
    tnh = T("tnh", [128, 2, cw])
    nc.vector.tensor_tensor(out=tnh.bitcast(U32), in0=uu.bitcast(U32), in1=sgn, op=ALU.bitwise_xor)
    lin = tnh[:, 0, :]
    lgs = tnh[:, 1, :]

    # Vm = exp(min(R, clamp)) + G*(|R| - exp(...))
    if BATCH_EXP:
        er = eo[:, 2, :]
    else:
        minr = T("minr")
        nc.vector.tensor_scalar(out=minr, in0=R, scalar1=LOG_CLAMP, scalar2=None, op0=ALU.min)
        er = T("er")
        nc.scalar.activation(er, minr, ACTF.Exp)
    ar = T("ar")
    nc.vector.tensor_scalar(out=ar.bitcast(U32), in0=R.bitcast(U32),
                            scalar1=0x7FFFFFFF, scalar2=None, op0=ALU.bitwise_and)
    d1 = T("d1")
    nc.vector.tensor_tensor(out=d1, in0=ar, in1=er, op=ALU.subtract)
    d2 = T("d2")
    nc.vector.tensor_tensor(out=d2, in0=G_s, in1=d1, op=ALU.mult)
    vm = T("vm")
    nc.vector.tensor_tensor(out=vm, in0=er, in1=d2, op=ALU.add)

    # Vs = log_sign + G*(lin_sign - log_sign)
    e1 = T("e1")
    nc.vector.tensor_tensor(out=e1, in0=lin, in1=lgs, op=ALU.subtract)
    e2 = T("e2")
    nc.vector.tensor_tensor(out=e2, in0=G_s, in1=e1, op=ALU.mult)
    vs = T("vs")
    nc.vector.tensor_tensor(out=vs, in0=lgs, in1=e2, op=ALU.add)

    if s == INTER - 1:
        nc.vector.tensor_tensor(out=st["OUT"], in0=vs, in1=vm, op=ALU.mult)
    else:
        idx = INIT_SLOTS + s
        sgnew = T("sgnew")
        nc.vector.tensor_tensor(out=sgnew, in0=vs, in1=vm, op=ALU.mult)
        vmc = T("vmc")
        nc.vector.tensor_scalar(out=vmc, in0=vm, scalar1=1e-12, scalar2=None, op0=ALU.max)
        nc.scalar.activation(LOGMAG[:, :, idx], vmc, ACTF.Ln)
        nc.vector.tensor_tensor(out=DIFF[:, :, idx], in0=sgnew,
                                in1=LOGMAG[:, :, idx], op=ALU.subtract)
        pv_next = ns.tile([128, cw], F32, tag=f"pv{sfx}")
        nc.vector.tensor_tensor(out=pv_next, in0=st["pv"], in1=vs, op=ALU.mult)
        st["pv"] = pv_next


def _build_looped(loop_n, parts="all"):
    """Timing build: body wrapped in a hardware For_i loop (loop_n reps)."""
    nc = bacc.Bacc("TRN2", target_bir_lowering=False, debug=False)

    hf_d = nc.dram_tensor("hf", [NTILE, H, 256], F16, kind="ExternalInput")
    wt_d = nc.dram_tensor("wt", [H, 2 * NF], F16, kind="ExternalInput")
    bias_d = nc.dram_tensor("bias", [1, 2 * NF], F16, kind="ExternalInput")
    out_d = nc.dram_tensor("out", [128, NTILE], F32, kind="ExternalOutput")

    with tile.TileContext(nc) as tc:
        with tc.tile_pool(name="consts", bufs=1) as consts, \
             tc.tile_pool(name="hfp", bufs=3) as hfp, \
             tc.tile_pool(name="evp", bufs=3) as evp, \
             tc.tile_pool(name="ns", bufs=2) as ns, \
             tc.tile_pool(name="pp", bufs=3, space="PSUM") as pp:

            wt_sb = consts.tile([128, KCH, 2 * NF], F16)
            nc.sync.dma_start(out=wt_sb, in_=wt_d.rearrange("(k p) f -> p k f", p=128))
            bias_sb = consts.tile([1, 2 * NF], F16)
            nc.sync.dma_start(out=bias_sb, in_=bias_d[:, :])
            ones = consts.tile([1, 128], F16)
            nc.vector.memset(ones, 1.0)

            with tc.For_i(0, loop_n):
                _emit_body(nc, tc, consts, hfp, evp, ns, pp,
                           hf_d, wt_sb, bias_sb, ones, out_d, parts=parts)

    nc.compile()
    return nc


def _get_nc():
    if "nc" not in _CACHE:
        _CACHE["nc"] = _build()
    return _CACHE["nc"]


def _prep_inputs(hidden, W_init, b_init, W_op, b_op, W_gate, b_gate):
    hidden = np.ascontiguousarray(np.asarray(hidden, np.float32)).reshape(NTOK, H)
    Wcat = np.concatenate([np.asarray(W_init, np.float32),
                           np.asarray(W_op, np.float32),
                           np.asarray(W_gate, np.float32)], axis=0)   # [168, H]
    bcat = np.concatenate([np.asarray(b_init, np.float32),
                           np.asarray(b_op, np.float32),
                           np.asarray(b_gate, np.float32)])           # [168]

    WT = np.ascontiguousarray(Wcat.T)                                  # [H, 168]
    Wh = WT.astype(np.float16)
    Wl = ((WT - Wh.astype(np.float32)) * SCL).astype(np.float16)
    wt = np.concatenate([Wh, Wl], axis=1)                              # [H, 336]

    bh = bcat.astype(np.float16)
    bl = ((bcat - bh.astype(np.float32)) * SCL).astype(np.float16)
    bias = np.concatenate([bh, bl])[None, :]                           # [1, 336]

    in_maps = []
    for c in range(NCORES):
        shard = hidden[c * TPC:(c + 1) * TPC]                          # [2048, H]
        hT = np.ascontiguousarray(shard.T)                             # [H, 2048]
        fh = hT.astype(np.float16)
        fl = ((hT - fh.astype(np.float32)) * SCL).astype(np.float16)
        # [NTILE, H, 256]: per tile i, [h, 0:128] = hi of tokens, [h, 128:256] = lo
        comb = np.empty((NTILE, H, 256), np.float16)
        for i in range(NTILE):
            comb[i, :, 0:128] = fh[:, i * 128:(i + 1) * 128]
            comb[i, :, 128:256] = fl[:, i * 128:(i + 1) * 128]
        in_maps.append({"hf": comb, "wt": wt, "bias": bias})
    return in_maps


def _run(in_maps, **kwargs):
    nc = _get_nc()
    return run_bass_kernel_spmd(nc, in_maps, core_ids=list(range(NCORES)), **kwargs)


def _assemble(results):
    out = np.empty((NTOK,), np.float32)
    for c in range(NCORES):
        out[c * TPC:(c + 1) * TPC] = results[c]["out"].T.reshape(TPC)
    return out.reshape(B, T)


def kernel(**inputs):
    in_maps = _prep_inputs(**inputs)
    res = _run(in_maps)
    return _assemble(res.results)


def kernel_traced(**inputs):
    """Like kernel() but with NTFF tracing; returns (output, BassKernelResults)."""
    in_maps = _prep_inputs(**inputs)
    res = _run(in_maps, trace=True)
    return _assemble(res.results), res



# revision 3
# speedup vs baseline: 1079.4047x; 2.4514x over previous
"""Trainium2 Bass kernel v2.1 for nn_NewDAGExecutor (plan matmul + 8-step DAG).

Strategy (8 cores, data-parallel over 16384 tokens, 2048 tokens/core):
  - Host: per-core token shard transposed to [H, tok], split fp32 into exact
    fp16 hi/lo (lo scaled 2^11), pre-swizzled to [p, tile, k, 256] so every
    DMA is a fully-contiguous per-partition transfer. First transfers are
    small (1-2 tiles) so the PE starts ~4us in; later ones are 4-tile.
  - Device: 3-pass fp16 matmul per token tile (hi*[Wh|Wl] N=336 + lo*Wh into
    the hi*Wl columns), bias via ones-matmul, combine P1 + 2^-11*P2.
  - DAG: 4 token chunks of 4 tiles; the first two run under the matmul
    phase, the last two interleave their serial step chains in the tail.
    All transcendentals via the single natural_log_exp table set (tanh and
    sigmoid computed from exp on DVE). LMD holds [log_mag; signed] so
    R = (1-G)*r1 + G*r2 via a precomputed [1-G; G] pair; V = Y*(1-G)+X*G.
    Sign-product |O|+1 factors are masked and batched across all 8 steps.
"""

import numpy as np

import concourse.bacc as bacc
import concourse.bass as bass
import concourse.tile as tile
import concourse.mybir as mybir
from concourse.bass_utils import run_bass_kernel_spmd

# Pin ln/exp to the natural_log_exp_and_others table set so the whole kernel
# needs exactly one ACT table load.
_ORIG_GAT = bacc.get_activation_tables


def _pinned_activation_tables(arch):
    tables = _ORIG_GAT(arch)
    LN = mybir.ActivationFunctionType.Ln
    EXP = mybir.ActivationFunctionType.Exp
    for name, funcs in tables.items():
        if name != "natural_log_exp_and_others":
            funcs.discard(LN)
            funcs.discard(EXP)
    return tables


bacc.get_activation_tables = _pinned_activation_tables

F32 = mybir.dt.float32
F16 = mybir.dt.float16
U32 = mybir.dt.uint32
ALU = mybir.AluOpType
ACTF = mybir.ActivationFunctionType
AXX = mybir.AxisListType.X

NCORES = 8
B, T, H = 4, 4096, 2048
NTOK = B * T                    # 16384
TPC = NTOK // NCORES            # 2048 tokens per core
NTILE = TPC // 128              # 16 token tiles per core
KCH = H // 128                  # 16 contraction chunks
NN = 16                         # DAG nodes
INTER = 8                       # steps
INIT_SLOTS = 8
NF = 168                        # 32 init + 128 op + 8 gate
LOG_CLAMP = 23.026
SCL = 2048.0                    # 2^11 lo-part scale
ISCL = 1.0 / SCL
EXPCAP = 88.0                   # keep exp() finite

CH_SIZES = [8, 8]            # DAG chunks (tiles)
DMA_GROUPS = [[1, 1, 2, 4], [4, 4]]  # DMA transfer sizes within each chunk
TAIL_ILV = 1                    # interleave the last N chunks' DAG steps
POOL_OFFLOAD = False            # batched init products on the Pool engine

_CACHE = {}


def _declare(nc):
    # ACT bias=1e-12 (the Ln clip) needs a registered scalar const AP.
    t = nc.alloc_sbuf_tensor("const-float32-1em12", [128, 1], F32)
    nc.gpsimd.memset(t.ap(), 1e-12)
    nc.const_aps.aps[(F32, 1e-12)] = t.ap()
    nc.all_engine_barrier()
    hf_d = nc.dram_tensor("hf", [128, NTILE, KCH, 256], F16, kind="ExternalInput")
    wt_d = nc.dram_tensor("wt", [128, KCH, 2 * NF], F16, kind="ExternalInput")
    bias_d = nc.dram_tensor("bias", [1, 2 * NF], F16, kind="ExternalInput")
    msk_d = nc.dram_tensor("msk", [128, INTER * NN], U32, kind="ExternalInput")
    out_d = nc.dram_tensor("out", [128, NTILE], F32, kind="ExternalOutput")
    return hf_d, wt_d, bias_d, msk_d, out_d


def _consts(nc, consts, wt_d, bias_d, msk_d):
    wt_sb = consts.tile([128, KCH, 2 * NF], F16)
    nc.sync.dma_start(out=wt_sb, in_=wt_d[:, :, :])
    bias_sb = consts.tile([1, 2 * NF], F16)
    nc.sync.dma_start(out=bias_sb, in_=bias_d[:, :])
    msk_sb = consts.tile([128, INTER * NN], U32)
    nc.sync.dma_start(out=msk_sb, in_=msk_d[:, :])
    ones = consts.tile([1, 128], F16)
    nc.vector.memset(ones, 1.0)
    return wt_sb, bias_sb, msk_sb, ones


def _pools(tc):
    return [tc.tile_pool(name="consts", bufs=1), tc.tile_pool(name="hfp", bufs=2),
            tc.tile_pool(name="evp", bufs=4), tc.tile_pool(name="ns", bufs=2),
            tc.tile_pool(name="pp", bufs=4, space="PSUM")]


def _build(repeats=1, parts="all"):
    nc = bacc.Bacc("TRN2", target_bir_lowering=False, debug=False)
    hf_d, wt_d, bias_d, msk_d, out_d = _declare(nc)
    with tile.TileContext(nc) as tc:
        p_consts, p_hfp, p_evp, p_ns, p_pp = _pools(tc)
        with p_consts as consts, p_hfp as hfp, p_evp as evp, p_ns as ns, p_pp as pp:
            cb = _consts(nc, consts, wt_d, bias_d, msk_d)
            for _ in range(repeats):
                _emit_body(nc, tc, consts, hfp, evp, ns, pp,
                           hf_d, *cb, out_d, parts=parts)
    nc.compile()
    return nc


def _build_looped(loop_n, parts="all"):
    nc = bacc.Bacc("TRN2", target_bir_lowering=False, debug=False)
    hf_d, wt_d, bias_d, msk_d, out_d = _declare(nc)
    with tile.TileContext(nc) as tc:
        p_consts, p_hfp, p_evp, p_ns, p_pp = _pools(tc)
        with p_consts as consts, p_hfp as hfp, p_evp as evp, p_ns as ns, p_pp as pp:
            cb = _consts(nc, consts, wt_d, bias_d, msk_d)
            with tc.For_i(0, loop_n):
                _emit_body(nc, tc, consts, hfp, evp, ns, pp,
                           hf_d, *cb, out_d, parts=parts)
    nc.compile()
    return nc


def _emit_body(nc, tc, consts, hfp, evp, ns, pp, hf_d, wt_sb, bias_sb, msk_sb,
               ones, out_d, parts="all"):
    plan = consts.tile([128, NTILE, NF], F32, tag="plan", name="plan")
    bounds = np.cumsum([0] + CH_SIZES)
    chunks = [(int(bounds[c]), int(bounds[c + 1]), chr(65 + c))
              for c in range(len(CH_SIZES))]

    def emit_dag(chlist, pool=False):
        sts = {}
        for lo_t, hi_t, sfx in chlist:
            sts[sfx] = _dag_init(nc, consts, ns, plan, msk_sb, lo_t, hi_t, sfx)
        for s in range(INTER):
            for lo_t, hi_t, sfx in chlist:
                _dag_step(nc, ns, plan, sts[sfx], lo_t, hi_t, sfx, s, pool=pool)
        for lo_t, hi_t, sfx in chlist:
            nc.sync.dma_start(out=out_d[:, lo_t:hi_t], in_=sts[sfx]["OUT"])

    n_ilv = min(TAIL_ILV, len(chunks))
    if parts == "dag":
        nc.vector.memset(plan, 0.5)
        for ch in chunks[:len(chunks) - n_ilv]:
            emit_dag([ch])
        emit_dag(chunks[len(chunks) - n_ilv:], pool=POOL_OFFLOAD)
        return

    for ci, (lo_t, hi_t, sfx) in enumerate(chunks):
        t0 = lo_t
        for gsz in DMA_GROUPS[ci]:
            hfg = hfp.tile([128, gsz, KCH, 256], F16, tag=f"hfg{gsz}",
                           name=f"hfg{gsz}")
            nc.sync.dma_start(out=hfg, in_=hf_d[:, t0:t0 + gsz])
            for t in range(gsz):
                i = t0 + t
                p12 = pp.tile([128, 2 * NF], F32, tag="p12", name="p12")
                nc.tensor.matmul(p12, ones[:, :], bias_sb[:, :],
                                 start=True, stop=False)
                for k in range(KCH):
                    hi = hfg[:, t, k, 0:128]
                    lo = hfg[:, t, k, 128:256]
                    nc.tensor.matmul(p12, hi, wt_sb[:, k, :],
                                     start=False, stop=False)
                    nc.tensor.matmul(p12[:, NF:2 * NF], lo, wt_sb[:, k, 0:NF],
                                     start=False, stop=(k == KCH - 1),
                                     skip_group_check=True)
                ev = evp.tile([128, NF], F32, tag="ev", name="ev")
                nc.scalar.activation(ev, p12[:, NF:2 * NF], ACTF.Copy,
                                     bias=0.0, scale=ISCL)
                nc.vector.tensor_tensor(out=plan[:, i, :], in0=p12[:, 0:NF],
                                        in1=ev, op=ALU.add)
            t0 += gsz
        if parts != "mm" and ci < len(chunks) - n_ilv:
            emit_dag([chunks[ci]])
    if parts == "mm":
        nc.sync.dma_start(out=out_d[:, :], in_=plan[:, :, 0])
        return
    emit_dag(chunks[len(chunks) - n_ilv:], pool=POOL_OFFLOAD)


def _dag_init(nc, consts, ns, plan, msk_sb, lo_t, hi_t, sfx):
    cw = hi_t - lo_t
    PLAN = plan[:, lo_t:hi_t, :]
    st = {}
    GP = st["GP"] = consts.tile([128, cw, 2, INTER], F32, tag=f"GP{sfx}",
                                name=f"GP{sfx}")
    G = st["G"] = GP[:, :, 1, :]
    VSIGN = consts.tile([128, cw, NN], F32, tag=f"VSIGN{sfx}", name=f"VSIGN{sfx}")
    LMD = st["LMD"] = consts.tile([128, 2, cw, NN], F32, tag=f"LMD{sfx}",
                                  name=f"LMD{sfx}")
    LOGMAG = LMD[:, 0, :, :]
    SIGNED = LMD[:, 1, :, :]
    PVX = st["PVX"] = consts.tile([128, cw, INTER], F32, tag=f"PVX{sfx}",
                                  name=f"PVX{sfx}")
    st["OUT"] = consts.tile([128, cw], F32, tag=f"OUT{sfx}", name=f"OUT{sfx}")

    # tanh(x) = 1 - 2/(e^{2x}+1); sigmoid(x) = 1/(e^{-x}+1): one batched
    # +1/recip pipeline over [e^{2x} | e^{-xg}] (24 cols per token).
    E24 = ns.tile([128, cw, NN + INTER], F32, tag=f"E24{sfx}", name=f"E24{sfx}")
    nc.scalar.activation(E24[:, :, 0:NN], PLAN[:, :, 16:32], ACTF.Exp,
                         bias=0.0, scale=2.0)
    nc.scalar.activation(E24[:, :, NN:NN + INTER], PLAN[:, :, 160:168], ACTF.Exp,
                         bias=0.0, scale=-1.0)
    nc.vector.tensor_scalar(out=E24, in0=E24, scalar1=1.0, scalar2=None,
                            op0=ALU.add)
    rg = ns.tile([128, cw, NN + INTER], F32, tag=f"rg{sfx}", name=f"rg{sfx}")
    nc.vector.reciprocal(out=rg, in_=E24)
    nc.vector.tensor_scalar(out=VSIGN, in0=rg[:, :, 0:NN], scalar1=-2.0,
                            scalar2=1.0, op0=ALU.mult, op1=ALU.add)
    nc.vector.tensor_copy(out=GP[:, :, 1, :], in_=rg[:, :, NN:NN + INTER])
    nc.vector.tensor_scalar(out=GP[:, :, 0, :], in0=GP[:, :, 1, :], scalar1=-1.0,
                            scalar2=1.0, op0=ALU.mult, op1=ALU.add)

    # V_mag = |init_raw| (unclipped, as the reference); Ln carries the 1e-12
    VMAG = ns.tile([128, cw, NN], F32, tag=f"VMAG{sfx}", name=f"VMAG{sfx}")
    nc.vector.tensor_scalar(out=VMAG.bitcast(U32), in0=PLAN[:, :, 0:16].bitcast(U32),
                            scalar1=0x7FFFFFFF, scalar2=None, op0=ALU.bitwise_and)
    nc.scalar.activation(LOGMAG, VMAG, ACTF.Ln, bias=1e-12, scale=1.0)
    nc.vector.tensor_tensor(out=SIGNED, in0=VSIGN, in1=VMAG, op=ALU.mult)

    # pv = prod V_sign[0:8]
    pva = ns.tile([128, cw, 4], F32, tag=f"pva{sfx}", name=f"pva{sfx}")
    nc.vector.tensor_tensor(out=pva, in0=VSIGN[:, :, 0:4], in1=VSIGN[:, :, 4:8],
                            op=ALU.mult)
    pvb = ns.tile([128, cw, 2], F32, tag=f"pvb{sfx}", name=f"pvb{sfx}")
    nc.vector.tensor_tensor(out=pvb, in0=pva[:, :, 0:2], in1=pva[:, :, 2:4],
                            op=ALU.mult)
    pv = ns.tile([128, cw], F32, tag=f"pv{sfx}", name=f"pv{sfx}")
    nc.vector.tensor_tensor(out=pv, in0=pvb[:, :, 0], in1=pvb[:, :, 1], op=ALU.mult)
    st["pv"] = pv

    # PVT[s] = prod_{j>=8+s} V_sign_init[j]  (suffix products)
    PVT = ns.tile([128, cw, INTER], F32, tag=f"PVT{sfx}", name=f"PVT{sfx}")
    nc.vector.tensor_copy(out=PVT[:, :, INTER - 1], in_=VSIGN[:, :, NN - 1])
    for j in range(INTER - 2, -1, -1):
        nc.vector.tensor_tensor(out=PVT[:, :, j], in0=PVT[:, :, j + 1],
                                in1=VSIGN[:, :, 8 + j], op=ALU.mult)

    # Masked |O|+1 factors for every step at once, then product tree:
    # T16[:, :, s] = prod_j (mask(s,j)*|O_sj| + 1)
    OB = ns.tile([128, cw, INTER, NN], F32, tag=f"OB{sfx}", name=f"OB{sfx}")
    oin = bass.AP(tensor=PLAN.tensor, offset=PLAN.offset + 32,
                  ap=[PLAN.ap[0], PLAN.ap[1], [NN, INTER], [1, NN]])
    mbc = bass.AP(tensor=msk_sb.tensor, offset=msk_sb.offset,
                  ap=[msk_sb.ap[0], [0, cw], [NN, INTER], [1, NN]])
    eng = nc.gpsimd if POOL_OFFLOAD else nc.vector
    nc.vector.tensor_tensor(out=OB.bitcast(U32), in0=oin.bitcast(U32), in1=mbc,
                            op=ALU.bitwise_and)
    eng.tensor_scalar(out=OB, in0=OB, scalar1=1.0, scalar2=None, op0=ALU.add)
    t8 = ns.tile([128, cw, INTER, 8], F32, tag=f"t8{sfx}", name=f"t8{sfx}")
    eng.tensor_tensor(out=t8, in0=OB[:, :, :, 0:8], in1=OB[:, :, :, 8:16],
                      op=ALU.mult)
    t4 = ns.tile([128, cw, INTER, 4], F32, tag=f"t4{sfx}", name=f"t4{sfx}")
    eng.tensor_tensor(out=t4, in0=t8[:, :, :, 0:4], in1=t8[:, :, :, 4:8],
                      op=ALU.mult)
    t2 = ns.tile([128, cw, INTER, 2], F32, tag=f"t2{sfx}", name=f"t2{sfx}")
    eng.tensor_tensor(out=t2, in0=t4[:, :, :, 0:2], in1=t4[:, :, :, 2:4],
                      op=ALU.mult)
    t16 = ns.tile([128, cw, INTER], F32, tag=f"t16{sfx}", name=f"t16{sfx}")
    eng.tensor_tensor(out=t16, in0=t2[:, :, :, 0], in1=t2[:, :, :, 1],
                      op=ALU.mult)
    eng.tensor_tensor(out=PVX, in0=t16, in1=PVT, op=ALU.mult)
    return st


def _dag_step(nc, ns, plan, st, lo_t, hi_t, sfx, s, pool=False):
    cw = hi_t - lo_t
    PLAN = plan[:, lo_t:hi_t, :]
    G, GP, LMD, PVX = st["G"], st["GP"], st["LMD"], st["PVX"]
    LOGMAG = LMD[:, 0, :, :]
    v = INIT_SLOTS + s
    c0 = 32 + NN * s

    def T(nm, shape=None, dt=F32):
        return ns.tile(shape or [128, cw], dt, tag=f"{nm}{sfx}", name=f"{nm}{sfx}")

    # r12 over the prefix (cols < vp); the newest column feeds R via the
    # Rpartial/q split so only q and R sit downstream of the previous Ln.
    vp = v - 1 if s > 0 else v
    m12 = T("m12", [128, cw, 2, NN])
    ob = bass.AP(tensor=PLAN.tensor, offset=PLAN.offset + c0,
                 ap=[PLAN.ap[0], PLAN.ap[1], [0, 2], [1, vp]])
    lv = bass.AP(tensor=LMD.tensor, offset=LMD.offset,
                 ap=[LMD.ap[0], LMD.ap[2], LMD.ap[1], [1, vp]])
    nc.vector.tensor_tensor(out=m12[:, :, :, :vp], in0=ob, in1=lv, op=ALU.mult)
    r12 = T("r12", [128, cw, 2])
    nc.vector.tensor_reduce(out=r12, in_=m12[:, :, :, :vp], op=ALU.add, axis=AXX)
    u2 = T("u2", [128, cw, 2])
    nc.vector.tensor_tensor(out=u2, in0=r12, in1=GP[:, :, :, s], op=ALU.mult)
    tmp32 = T("tmp32", [128, 2, cw])
    R = tmp32[:, 0, :]
    SP = tmp32[:, 1, :]
    rp = T("rp") if s > 0 else R
    nc.vector.tensor_tensor(out=rp, in0=u2[:, :, 0], in1=u2[:, :, 1], op=ALU.add)
    if vp != v:
        nnode = v - 1
        onew = PLAN[:, :, c0 + nnode]
        gms = bass.AP(tensor=GP.tensor, offset=GP.offset + s,
                      ap=[GP.ap[0], [2 * INTER, cw]])
        on = T("on")
        nc.vector.tensor_tensor(out=on, in0=onew, in1=gms, op=ALU.mult)
        osg = T("osg")
        nc.vector.tensor_tensor(out=osg, in0=onew, in1=LMD[:, 1, :, nnode],
                                op=ALU.mult)
        gos = T("gos")
        nc.vector.tensor_tensor(out=gos, in0=GP[:, :, 1, s], in1=osg, op=ALU.mult)
        rpart = T("rpart")
        nc.vector.tensor_tensor(out=rpart, in0=rp, in1=gos, op=ALU.add)
        q = T("q")
        nc.vector.tensor_tensor(out=q, in0=on, in1=LMD[:, 0, :, nnode], op=ALU.mult)
        nc.vector.tensor_tensor(out=R, in0=rpart, in1=q, op=ALU.add)
    nc.vector.tensor_tensor(out=SP, in0=st["pv"], in1=PVX[:, :, s], op=ALU.mult)

    # U slots: 0=er 1=etR 2=etSP 3=lin 4=lgs 5=ar
    MM3 = T("MM3", [128, 3, cw])
    nc.vector.tensor_scalar(out=MM3[:, 0, :], in0=R, scalar1=LOG_CLAMP,
                            scalar2=None, op0=ALU.min)
    nc.vector.tensor_scalar(out=MM3[:, 1:3, :], in0=tmp32, scalar1=2.0e4,
                            scalar2=EXPCAP, op0=ALU.mult, op1=ALU.min)
    U = T("U", [128, 6, cw])
    nc.scalar.activation(U[:, 0:3, :], MM3, ACTF.Exp)
    den = T("den", [128, 2, cw])
    nc.vector.tensor_scalar(out=den, in0=U[:, 1:3, :], scalar1=1.0, scalar2=None,
                            op0=ALU.add)
    rd = T("rd", [128, 2, cw])
    nc.vector.reciprocal(out=rd, in_=den)
    nc.vector.tensor_scalar(out=U[:, 3:5, :], in0=rd, scalar1=-2.0, scalar2=1.0,
                            op0=ALU.mult, op1=ALU.add)
    nc.vector.tensor_scalar(out=U[:, 5, :].bitcast(U32), in0=R.bitcast(U32),
                            scalar1=0x7FFFFFFF, scalar2=None, op0=ALU.bitwise_and)

    # V = Y*(1-G) + X*G with X=[|R|, lin], Y=[er, lgs] -> [vm, vs]
    X = bass.AP(tensor=U.tensor, offset=U.offset + 5 * cw,
                ap=[U.ap[0], [-2 * cw, 2], [1, cw]])
    Y = bass.AP(tensor=U.tensor, offset=U.offset, ap=[U.ap[0], [4 * cw, 2], [1, cw]])
    gmb = bass.AP(tensor=GP.tensor, offset=GP.offset + s,
                  ap=[GP.ap[0], [0, 2], [2 * INTER, cw]])  # (1-G_s) bcast pair
    gb = bass.AP(tensor=G.tensor, offset=G.offset + s,
                 ap=[G.ap[0], [0, 2], [2 * INTER, cw]])
    ya = T("ya", [128, 2, cw])
    nc.vector.tensor_tensor(out=ya, in0=Y, in1=gmb, op=ALU.mult)
    xb = T("xb", [128, 2, cw])
    nc.vector.tensor_tensor(out=xb, in0=X, in1=gb, op=ALU.mult)
    V = T("V", [128, 2, cw])
    nc.vector.tensor_tensor(out=V, in0=ya, in1=xb, op=ALU.add)

    if s == INTER - 1:
        nc.vector.tensor_tensor(out=st["OUT"], in0=V[:, 0, :], in1=V[:, 1, :],
                                op=ALU.mult)
        return
    idx = INIT_SLOTS + s
    nc.scalar.activation(LOGMAG[:, :, idx], V[:, 0, :], ACTF.Ln, bias=1e-12,
                         scale=1.0)
    nc.vector.tensor_tensor(out=LMD[:, 1, :, idx], in0=V[:, 0, :], in1=V[:, 1, :],
                            op=ALU.mult)
    pvn = ns.tile([128, cw], F32, tag=f"pv{sfx}", name=f"pv{sfx}")
    nc.vector.tensor_tensor(out=pvn, in0=st["pv"], in1=V[:, 1, :], op=ALU.mult)
    st["pv"] = pvn


def _get_nc():
    if "nc" not in _CACHE:
        _CACHE["nc"] = _build()
    return _CACHE["nc"]


def _mask_host():
    msk = np.zeros((128, INTER * NN), np.uint32)
    for s in range(INTER):
        msk[:, s * NN: s * NN + INIT_SLOTS + s] = 0x7FFFFFFF
    return msk


def _prep_inputs(hidden, W_init, b_init, W_op, b_op, W_gate, b_gate):
    hidden = np.ascontiguousarray(np.asarray(hidden, np.float32)).reshape(NTOK, H)
    Wcat = np.concatenate([np.asarray(W_init, np.float32),
                           np.asarray(W_op, np.float32),
                           np.asarray(W_gate, np.float32)], axis=0)   # [168, H]
    bcat = np.concatenate([np.asarray(b_init, np.float32),
                           np.asarray(b_op, np.float32),
                           np.asarray(b_gate, np.float32)])           # [168]

    WT = np.ascontiguousarray(Wcat.T)                                  # [H, 168]
    Wh = WT.astype(np.float16)
    Wl = ((WT - Wh.astype(np.float32)) * SCL).astype(np.float16)
    wt = np.concatenate(
        [Wh.reshape(KCH, 128, NF).transpose(1, 0, 2),
         Wl.reshape(KCH, 128, NF).transpose(1, 0, 2)], axis=2)
    wt = np.ascontiguousarray(wt)

    bh = bcat.astype(np.float16)
    bl = ((bcat - bh.astype(np.float32)) * SCL).astype(np.float16)
    bias = np.concatenate([bh, bl])[None, :]                           # [1, 336]
    msk = _mask_host()

    in_maps = []
    for c in range(NCORES):
        shard = hidden[c * TPC:(c + 1) * TPC]                          # [2048, H]
        hT = np.ascontiguousarray(shard.T)                             # [H, 2048]
        fh = hT.astype(np.float16)
        fl = ((hT - fh.astype(np.float32)) * SCL).astype(np.float16)
        comb = np.empty((128, NTILE, KCH, 256), np.float16)
        comb[..., 0:128] = fh.reshape(KCH, 128, NTILE, 128).transpose(1, 2, 0, 3)
        comb[..., 128:256] = fl.reshape(KCH, 128, NTILE, 128).transpose(1, 2, 0, 3)
        in_maps.append({"hf": comb, "wt": wt, "bias": bias, "msk": msk})
    return in_maps


def _run(in_maps, **kwargs):
    nc = _get_nc()
    return run_bass_kernel_spmd(nc, in_maps, core_ids=list(range(NCORES)), **kwargs)


def _assemble(results):
    out = np.empty((NTOK,), np.float32)
    for c in range(NCORES):
        out[c * TPC:(c + 1) * TPC] = results[c]["out"].T.reshape(TPC)
    return out.reshape(B, T)


def kernel(**inputs):
    in_maps = _prep_inputs(**inputs)
    res = _run(in_maps)
    return _assemble(res.results)


def kernel_traced(**inputs):
    in_maps = _prep_inputs(**inputs)
    res = _run(in_maps, trace=True)
    return _assemble(res.results), res


# revision 6
# speedup vs baseline: 1096.1111x; 1.0155x over previous
"""Trainium2 Bass kernel v2.1 for nn_NewDAGExecutor (plan matmul + 8-step DAG).

Strategy (8 cores, data-parallel over 16384 tokens, 2048 tokens/core):
  - Host: per-core token shard transposed to [H, tok], split fp32 into exact
    fp16 hi/lo (lo scaled 2^11), pre-swizzled to [p, tile, k, 256] so every
    DMA is a fully-contiguous per-partition transfer. First transfers are
    small (1-2 tiles) so the PE starts ~4us in; later ones are 4-tile.
  - Device: 3-pass fp16 matmul per token tile (hi*[Wh|Wl] N=336 + lo*Wh into
    the hi*Wl columns), bias via ones-matmul, combine P1 + 2^-11*P2.
  - DAG: 4 token chunks of 4 tiles; the first two run under the matmul
    phase, the last two interleave their serial step chains in the tail.
    All transcendentals via the single natural_log_exp table set (tanh and
    sigmoid computed from exp on DVE). LMD holds [log_mag; signed] so
    R = (1-G)*r1 + G*r2 via a precomputed [1-G; G] pair; V = Y*(1-G)+X*G.
    Sign-product |O|+1 factors are masked and batched across all 8 steps.
"""

import numpy as np

import concourse.bacc as bacc
import concourse.bass as bass
import concourse.tile as tile
import concourse.mybir as mybir
from concourse.bass_utils import run_bass_kernel_spmd

# Pin ln/exp to the natural_log_exp_and_others table set so the whole kernel
# needs exactly one ACT table load.
_ORIG_GAT = bacc.get_activation_tables


def _pinned_activation_tables(arch):
    tables = _ORIG_GAT(arch)
    LN = mybir.ActivationFunctionType.Ln
    EXP = mybir.ActivationFunctionType.Exp
    for name, funcs in tables.items():
        if name != "natural_log_exp_and_others":
            funcs.discard(LN)
            funcs.discard(EXP)
    return tables


bacc.get_activation_tables = _pinned_activation_tables

F32 = mybir.dt.float32
F16 = mybir.dt.float16
U32 = mybir.dt.uint32
ALU = mybir.AluOpType
ACTF = mybir.ActivationFunctionType
AXX = mybir.AxisListType.X

NCORES = 8
B, T, H = 4, 4096, 2048
NTOK = B * T                    # 16384
TPC = NTOK // NCORES            # 2048 tokens per core
NTILE = TPC // 128              # 16 token tiles per core
KCH = H // 128                  # 16 contraction chunks
NN = 16                         # DAG nodes
INTER = 8                       # steps
INIT_SLOTS = 8
NF = 168                        # 32 init + 128 op + 8 gate
LOG_CLAMP = 23.026
SCL = 2048.0                    # 2^11 lo-part scale
ISCL = 1.0 / SCL
EXPCAP = 88.0                   # keep exp() finite

CH_SIZES = [8, 8]            # DAG chunks (tiles)
DMA_GROUPS = [[1, 1, 2, 4], [4, 4]]  # DMA transfer sizes within each chunk
TAIL_ILV = 1                    # interleave the last N chunks' DAG steps
POOL_OFFLOAD = False            # batched init products on the Pool engine

_CACHE = {}


def _declare(nc):
    # ACT bias=1e-12 (the Ln clip) needs a registered scalar const AP.
    t = nc.alloc_sbuf_tensor("const-float32-1em12", [128, 1], F32)
    nc.gpsimd.memset(t.ap(), 1e-12)
    nc.const_aps.aps[(F32, 1e-12)] = t.ap()
    nc.all_engine_barrier()
    hf_d = nc.dram_tensor("hf", [128, NTILE, KCH, 256], F16, kind="ExternalInput")
    wt_d = nc.dram_tensor("wt", [128, KCH, 2 * NF], F16, kind="ExternalInput")
    bias_d = nc.dram_tensor("bias", [1, 2 * NF], F16, kind="ExternalInput")
    msk_d = nc.dram_tensor("msk", [128, INTER * NN], U32, kind="ExternalInput")
    out_d = nc.dram_tensor("out", [128, NTILE], F32, kind="ExternalOutput")
    return hf_d, wt_d, bias_d, msk_d, out_d


def _consts(nc, consts, wt_d, bias_d, msk_d):
    # Constants ride the scalar-issued HWDGE ring so the SP ring starts on
    # hidden prefetch immediately; wt is split so the first matmuls only
    # wait for its first k-chunks.
    bias_sb = consts.tile([1, 2 * NF], F16)
    nc.sync.dma_start(out=bias_sb, in_=bias_d[:, :])
    wt_sb = consts.tile([128, KCH, 2 * NF], F16)
    nc.sync.dma_start(out=wt_sb[:, 0:4, :], in_=wt_d[:, 0:4, :])
    nc.scalar.dma_start(out=wt_sb[:, 4:KCH, :], in_=wt_d[:, 4:KCH, :])
    msk_sb = consts.tile([128, INTER * NN], U32)
    nc.scalar.dma_start(out=msk_sb, in_=msk_d[:, :])
    ones = consts.tile([1, 128], F16)
    nc.vector.memset(ones, 1.0)
    return wt_sb, bias_sb, msk_sb, ones


def _pools(tc):
    return [tc.tile_pool(name="consts", bufs=1), tc.tile_pool(name="hfp", bufs=2),
            tc.tile_pool(name="evp", bufs=4), tc.tile_pool(name="ns", bufs=2),
            tc.tile_pool(name="pp", bufs=4, space="PSUM")]


def _build(repeats=1, parts="all"):
    nc = bacc.Bacc("TRN2", target_bir_lowering=False, debug=False)
    hf_d, wt_d, bias_d, msk_d, out_d = _declare(nc)
    with tile.TileContext(nc) as tc:
        p_consts, p_hfp, p_evp, p_ns, p_pp = _pools(tc)
        with p_consts as consts, p_hfp as hfp, p_evp as evp, p_ns as ns, p_pp as pp:
            cb = _consts(nc, consts, wt_d, bias_d, msk_d)
            for _ in range(repeats):
                _emit_body(nc, tc, consts, hfp, evp, ns, pp,
                           hf_d, *cb, out_d, parts=parts)
    nc.compile()
    return nc


def _build_looped(loop_n, parts="all"):
    nc = bacc.Bacc("TRN2", target_bir_lowering=False, debug=False)
    hf_d, wt_d, bias_d, msk_d, out_d = _declare(nc)
    with tile.TileContext(nc) as tc:
        p_consts, p_hfp, p_evp, p_ns, p_pp = _pools(tc)
        with p_consts as consts, p_hfp as hfp, p_evp as evp, p_ns as ns, p_pp as pp:
            cb = _consts(nc, consts, wt_d, bias_d, msk_d)
            with tc.For_i(0, loop_n):
                _emit_body(nc, tc, consts, hfp, evp, ns, pp,
                           hf_d, *cb, out_d, parts=parts)
    nc.compile()
    return nc


def _emit_body(nc, tc, consts, hfp, evp, ns, pp, hf_d, wt_sb, bias_sb, msk_sb,
               ones, out_d, parts="all"):
    plan = consts.tile([128, NTILE, NF], F32, tag="plan", name="plan")
    bounds = np.cumsum([0] + CH_SIZES)
    chunks = [(int(bounds[c]), int(bounds[c + 1]), chr(65 + c))
              for c in range(len(CH_SIZES))]

    def emit_dag(chlist, pool=False):
        sts = {}
        for lo_t, hi_t, sfx in chlist:
            sts[sfx] = _dag_init(nc, consts, ns, plan, msk_sb, lo_t, hi_t, sfx)
        for s in range(INTER):
            for lo_t, hi_t, sfx in chlist:
                _dag_step(nc, ns, plan, sts[sfx], lo_t, hi_t, sfx, s, pool=pool)
        for lo_t, hi_t, sfx in chlist:
            # scalar-issued HWDGE: keeps the SP queue free for input prefetch
            nc.scalar.dma_start(out=out_d[:, lo_t:hi_t], in_=sts[sfx]["OUT"])

    n_ilv = min(TAIL_ILV, len(chunks))
    if parts == "dag":
        nc.vector.memset(plan, 0.5)
        for ch in chunks[:len(chunks) - n_ilv]:
            emit_dag([ch])
        emit_dag(chunks[len(chunks) - n_ilv:], pool=POOL_OFFLOAD)
        return

    for ci, (lo_t, hi_t, sfx) in enumerate(chunks):
        t0 = lo_t
        for gsz in DMA_GROUPS[ci]:
            hfg = hfp.tile([128, gsz, KCH, 256], F16, tag=f"hfg{gsz}",
                           name=f"hfg{gsz}")
            nc.sync.dma_start(out=hfg, in_=hf_d[:, t0:t0 + gsz])
            for t in range(gsz):
                i = t0 + t
                p12 = pp.tile([128, 2 * NF], F32, tag="p12", name="p12")
                nc.tensor.matmul(p12, ones[:, :], bias_sb[:, :],
                                 start=True, stop=False)
                for k in range(KCH):
                    hi = hfg[:, t, k, 0:128]
                    lo = hfg[:, t, k, 128:256]
                    nc.tensor.matmul(p12, hi, wt_sb[:, k, :],
                                     start=False, stop=False)
                    nc.tensor.matmul(p12[:, NF:2 * NF], lo, wt_sb[:, k, 0:NF],
                                     start=False, stop=(k == KCH - 1),
                                     skip_group_check=True)
                ev = evp.tile([128, NF], F32, tag="ev", name="ev")
                nc.scalar.activation(ev, p12[:, NF:2 * NF], ACTF.Copy,
                                     bias=0.0, scale=ISCL)
                nc.vector.tensor_tensor(out=plan[:, i, :], in0=p12[:, 0:NF],
                                        in1=ev, op=ALU.add)
            t0 += gsz
        if parts != "mm" and ci < len(chunks) - n_ilv:
            emit_dag([chunks[ci]])
    if parts == "mm":
        nc.sync.dma_start(out=out_d[:, :], in_=plan[:, :, 0])
        return
    emit_dag(chunks[len(chunks) - n_ilv:], pool=POOL_OFFLOAD)


def _dag_init(nc, consts, ns, plan, msk_sb, lo_t, hi_t, sfx):
    cw = hi_t - lo_t
    PLAN = plan[:, lo_t:hi_t, :]
    st = {}
    GP = st["GP"] = consts.tile([128, cw, 2, INTER], F32, tag=f"GP{sfx}",
                                name=f"GP{sfx}")
    G = st["G"] = GP[:, :, 1, :]
    VSIGN = consts.tile([128, cw, NN], F32, tag=f"VSIGN{sfx}", name=f"VSIGN{sfx}")
    LMD = st["LMD"] = consts.tile([128, 2, cw, NN], F32, tag=f"LMD{sfx}",
                                  name=f"LMD{sfx}")
    LOGMAG = LMD[:, 0, :, :]
    SIGNED = LMD[:, 1, :, :]
    PVX = st["PVX"] = consts.tile([128, cw, INTER], F32, tag=f"PVX{sfx}",
                                  name=f"PVX{sfx}")
    st["OUT"] = consts.tile([128, cw], F32, tag=f"OUT{sfx}", name=f"OUT{sfx}")

    # tanh(x) = 1 - 2/(e^{2x}+1); sigmoid(x) = 1/(e^{-x}+1): one batched
    # +1/recip pipeline over [e^{2x} | e^{-xg}] (24 cols per token).
    E24 = ns.tile([128, cw, NN + INTER], F32, tag=f"E24{sfx}", name=f"E24{sfx}")
    nc.scalar.activation(E24[:, :, 0:NN], PLAN[:, :, 16:32], ACTF.Exp,
                         bias=0.0, scale=2.0)
    nc.scalar.activation(E24[:, :, NN:NN + INTER], PLAN[:, :, 160:168], ACTF.Exp,
                         bias=0.0, scale=-1.0)
    nc.vector.tensor_scalar(out=E24, in0=E24, scalar1=1.0, scalar2=None,
                            op0=ALU.add)
    rg = ns.tile([128, cw, NN + INTER], F32, tag=f"rg{sfx}", name=f"rg{sfx}")
    nc.vector.reciprocal(out=rg, in_=E24)
    nc.vector.tensor_scalar(out=VSIGN, in0=rg[:, :, 0:NN], scalar1=-2.0,
                            scalar2=1.0, op0=ALU.mult, op1=ALU.add)
    nc.vector.tensor_copy(out=GP[:, :, 1, :], in_=rg[:, :, NN:NN + INTER])
    nc.vector.tensor_scalar(out=GP[:, :, 0, :], in0=GP[:, :, 1, :], scalar1=-1.0,
                            scalar2=1.0, op0=ALU.mult, op1=ALU.add)

    # V_mag = |init_raw| (unclipped, as the reference); Ln carries the 1e-12
    VMAG = ns.tile([128, cw, NN], F32, tag=f"VMAG{sfx}", name=f"VMAG{sfx}")
    nc.vector.tensor_scalar(out=VMAG.bitcast(U32), in0=PLAN[:, :, 0:16].bitcast(U32),
                            scalar1=0x7FFFFFFF, scalar2=None, op0=ALU.bitwise_and)
    nc.scalar.activation(LOGMAG, VMAG, ACTF.Ln, bias=1e-12, scale=1.0)
    nc.vector.tensor_tensor(out=SIGNED, in0=VSIGN, in1=VMAG, op=ALU.mult)

    # pv = prod V_sign[0:8]
    pva = ns.tile([128, cw, 4], F32, tag=f"pva{sfx}", name=f"pva{sfx}")
    nc.vector.tensor_tensor(out=pva, in0=VSIGN[:, :, 0:4], in1=VSIGN[:, :, 4:8],
                            op=ALU.mult)
    pvb = ns.tile([128, cw, 2], F32, tag=f"pvb{sfx}", name=f"pvb{sfx}")
    nc.vector.tensor_tensor(out=pvb, in0=pva[:, :, 0:2], in1=pva[:, :, 2:4],
                            op=ALU.mult)
    pv = ns.tile([128, cw], F32, tag=f"pv{sfx}", name=f"pv{sfx}")
    nc.vector.tensor_tensor(out=pv, in0=pvb[:, :, 0], in1=pvb[:, :, 1], op=ALU.mult)
    st["pv"] = pv

    # PVT[s] = prod_{j>=8+s} V_sign_init[j]  (suffix products)
    PVT = ns.tile([128, cw, INTER], F32, tag=f"PVT{sfx}", name=f"PVT{sfx}")
    nc.vector.tensor_copy(out=PVT[:, :, INTER - 1], in_=VSIGN[:, :, NN - 1])
    for j in range(INTER - 2, -1, -1):
        nc.vector.tensor_tensor(out=PVT[:, :, j], in0=PVT[:, :, j + 1],
                                in1=VSIGN[:, :, 8 + j], op=ALU.mult)

    # Masked |O|+1 factors for every step at once, then product tree:
    # T16[:, :, s] = prod_j (mask(s,j)*|O_sj| + 1)
    OB = ns.tile([128, cw, INTER, NN], F32, tag=f"OB{sfx}", name=f"OB{sfx}")
    oin = bass.AP(tensor=PLAN.tensor, offset=PLAN.offset + 32,
                  ap=[PLAN.ap[0], PLAN.ap[1], [NN, INTER], [1, NN]])
    mbc = bass.AP(tensor=msk_sb.tensor, offset=msk_sb.offset,
                  ap=[msk_sb.ap[0], [0, cw], [NN, INTER], [1, NN]])
    eng = nc.gpsimd if POOL_OFFLOAD else nc.vector
    nc.vector.tensor_tensor(out=OB.bitcast(U32), in0=oin.bitcast(U32), in1=mbc,
                            op=ALU.bitwise_and)
    eng.tensor_scalar(out=OB, in0=OB, scalar1=1.0, scalar2=None, op0=ALU.add)
    t8 = ns.tile([128, cw, INTER, 8], F32, tag=f"t8{sfx}", name=f"t8{sfx}")
    eng.tensor_tensor(out=t8, in0=OB[:, :, :, 0:8], in1=OB[:, :, :, 8:16],
                      op=ALU.mult)
    t4 = ns.tile([128, cw, INTER, 4], F32, tag=f"t4{sfx}", name=f"t4{sfx}")
    eng.tensor_tensor(out=t4, in0=t8[:, :, :, 0:4], in1=t8[:, :, :, 4:8],
                      op=ALU.mult)
    t2 = ns.tile([128, cw, INTER, 2], F32, tag=f"t2{sfx}", name=f"t2{sfx}")
    eng.tensor_tensor(out=t2, in0=t4[:, :, :, 0:2], in1=t4[:, :, :, 2:4],
                      op=ALU.mult)
    t16 = ns.tile([128, cw, INTER], F32, tag=f"t16{sfx}", name=f"t16{sfx}")
    eng.tensor_tensor(out=t16, in0=t2[:, :, :, 0], in1=t2[:, :, :, 1],
                      op=ALU.mult)
    eng.tensor_tensor(out=PVX, in0=t16, in1=PVT, op=ALU.mult)
    return st


def _dag_step(nc, ns, plan, st, lo_t, hi_t, sfx, s, pool=False):
    cw = hi_t - lo_t
    PLAN = plan[:, lo_t:hi_t, :]
    G, GP, LMD, PVX = st["G"], st["GP"], st["LMD"], st["PVX"]
    LOGMAG = LMD[:, 0, :, :]
    v = INIT_SLOTS + s
    c0 = 32 + NN * s

    def T(nm, shape=None, dt=F32):
        return ns.tile(shape or [128, cw], dt, tag=f"{nm}{sfx}", name=f"{nm}{sfx}")

    # r12 over the prefix (cols < vp); the newest column feeds R via the
    # Rpartial/q split so only q and R sit downstream of the previous Ln.
    vp = v - 1 if s > 0 else v
    m12 = T("m12", [128, cw, 2, NN])
    ob = bass.AP(tensor=PLAN.tensor, offset=PLAN.offset + c0,
                 ap=[PLAN.ap[0], PLAN.ap[1], [0, 2], [1, vp]])
    lv = bass.AP(tensor=LMD.tensor, offset=LMD.offset,
                 ap=[LMD.ap[0], LMD.ap[2], LMD.ap[1], [1, vp]])
    nc.vector.tensor_tensor(out=m12[:, :, :, :vp], in0=ob, in1=lv, op=ALU.mult)
    r12 = T("r12", [128, cw, 2])
    nc.vector.tensor_reduce(out=r12, in_=m12[:, :, :, :vp], op=ALU.add, axis=AXX)
    u2 = T("u2", [128, cw, 2])
    nc.vector.tensor_tensor(out=u2, in0=r12, in1=GP[:, :, :, s], op=ALU.mult)
    tmp32 = T("tmp32", [128, 2, cw])
    R = tmp32[:, 0, :]
    SP = tmp32[:, 1, :]
    rp = T("rp") if s > 0 else R
    nc.vector.tensor_tensor(out=rp, in0=u2[:, :, 0], in1=u2[:, :, 1], op=ALU.add)
    if vp != v:
        nnode = v - 1
        onew = PLAN[:, :, c0 + nnode]
        gms = bass.AP(tensor=GP.tensor, offset=GP.offset + s,
                      ap=[GP.ap[0], [2 * INTER, cw]])
        on = T("on")
        nc.vector.tensor_tensor(out=on, in0=onew, in1=gms, op=ALU.mult)
        osg = T("osg")
        nc.vector.tensor_tensor(out=osg, in0=onew, in1=LMD[:, 1, :, nnode],
                                op=ALU.mult)
        gos = T("gos")
        nc.vector.tensor_tensor(out=gos, in0=GP[:, :, 1, s], in1=osg, op=ALU.mult)
        rpart = T("rpart")
        nc.vector.tensor_tensor(out=rpart, in0=rp, in1=gos, op=ALU.add)
        q = T("q")
        nc.vector.tensor_tensor(out=q, in0=on, in1=LMD[:, 0, :, nnode], op=ALU.mult)
        nc.vector.tensor_tensor(out=R, in0=rpart, in1=q, op=ALU.add)
    nc.vector.tensor_tensor(out=SP, in0=st["pv"], in1=PVX[:, :, s], op=ALU.mult)

    # U slots: 0=er 1=etR 2=etSP 3=lin 4=lgs 5=ar
    MM3 = T("MM3", [128, 3, cw])
    nc.vector.tensor_scalar(out=MM3[:, 0, :], in0=R, scalar1=LOG_CLAMP,
                            scalar2=None, op0=ALU.min)
    nc.vector.tensor_scalar(out=MM3[:, 1:3, :], in0=tmp32, scalar1=2.0e4,
                            scalar2=EXPCAP, op0=ALU.mult, op1=ALU.min)
    U = T("U", [128, 6, cw])
    nc.scalar.activation(U[:, 0:3, :], MM3, ACTF.Exp)
    den = T("den", [128, 2, cw])
    nc.vector.tensor_scalar(out=den, in0=U[:, 1:3, :], scalar1=1.0, scalar2=None,
                            op0=ALU.add)
    rd = T("rd", [128, 2, cw])
    nc.vector.reciprocal(out=rd, in_=den)
    nc.vector.tensor_scalar(out=U[:, 3:5, :], in0=rd, scalar1=-2.0, scalar2=1.0,
                            op0=ALU.mult, op1=ALU.add)
    nc.vector.tensor_scalar(out=U[:, 5, :].bitcast(U32), in0=R.bitcast(U32),
                            scalar1=0x7FFFFFFF, scalar2=None, op0=ALU.bitwise_and)

    # V = Y*(1-G) + X*G with X=[|R|, lin], Y=[er, lgs] -> [vm, vs]
    X = bass.AP(tensor=U.tensor, offset=U.offset + 5 * cw,
                ap=[U.ap[0], [-2 * cw, 2], [1, cw]])
    Y = bass.AP(tensor=U.tensor, offset=U.offset, ap=[U.ap[0], [4 * cw, 2], [1, cw]])
    gmb = bass.AP(tensor=GP.tensor, offset=GP.offset + s,
                  ap=[GP.ap[0], [0, 2], [2 * INTER, cw]])  # (1-G_s) bcast pair
    gb = bass.AP(tensor=G.tensor, offset=G.offset + s,
                 ap=[G.ap[0], [0, 2], [2 * INTER, cw]])
    ya = T("ya", [128, 2, cw])
    nc.vector.tensor_tensor(out=ya, in0=Y, in1=gmb, op=ALU.mult)
    xb = T("xb", [128, 2, cw])
    nc.vector.tensor_tensor(out=xb, in0=X, in1=gb, op=ALU.mult)
    V = T("V", [128, 2, cw])
    nc.vector.tensor_tensor(out=V, in0=ya, in1=xb, op=ALU.add)

    if s == INTER - 1:
        nc.vector.tensor_tensor(out=st["OUT"], in0=V[:, 0, :], in1=V[:, 1, :],
                                op=ALU.mult)
        return
    idx = INIT_SLOTS + s
    nc.scalar.activation(LOGMAG[:, :, idx], V[:, 0, :], ACTF.Ln, bias=1e-12,
                         scale=1.0)
    nc.vector.tensor_tensor(out=LMD[:, 1, :, idx], in0=V[:, 0, :], in1=V[:, 1, :],
                            op=ALU.mult)
    pvn = ns.tile([128, cw], F32, tag=f"pv{sfx}", name=f"pv{sfx}")
    nc.vector.tensor_tensor(out=pvn, in0=st["pv"], in1=V[:, 1, :], op=ALU.mult)
    st["pv"] = pvn


def _get_nc():
    if "nc" not in _CACHE:
        _CACHE["nc"] = _build()
    return _CACHE["nc"]


def _mask_host():
    msk = np.zeros((128, INTER * NN), np.uint32)
    for s in range(INTER):
        msk[:, s * NN: s * NN + INIT_SLOTS + s] = 0x7FFFFFFF
    return msk


def _prep_inputs(hidden, W_init, b_init, W_op, b_op, W_gate, b_gate):
    hidden = np.ascontiguousarray(np.asarray(hidden, np.float32)).reshape(NTOK, H)
    Wcat = np.concatenate([np.asarray(W_init, np.float32),
                           np.asarray(W_op, np.float32),
                           np.asarray(W_gate, np.float32)], axis=0)   # [168, H]
    bcat = np.concatenate([np.asarray(b_init, np.float32),
                           np.asarray(b_op, np.float32),
                           np.asarray(b_gate, np.float32)])           # [168]

    WT = np.ascontiguousarray(Wcat.T)                                  # [H, 168]
    Wh = WT.astype(np.float16)
    Wl = ((WT - Wh.astype(np.float32)) * SCL).astype(np.float16)
    wt = np.concatenate(
        [Wh.reshape(KCH, 128, NF).transpose(1, 0, 2),
         Wl.reshape(KCH, 128, NF).transpose(1, 0, 2)], axis=2)
    wt = np.ascontiguousarray(wt)

    bh = bcat.astype(np.float16)
    bl = ((bcat - bh.astype(np.float32)) * SCL).astype(np.float16)
    bias = np.concatenate([bh, bl])[None, :]                           # [1, 336]
    msk = _mask_host()

    in_maps = []
    for c in range(NCORES):
        shard = hidden[c * TPC:(c + 1) * TPC]                          # [2048, H]
        hT = np.ascontiguousarray(shard.T)                             # [H, 2048]
        fh = hT.astype(np.float16)
        fl = ((hT - fh.astype(np.float32)) * SCL).astype(np.float16)
        comb = np.empty((128, NTILE, KCH, 256), np.float16)
        comb[..., 0:128] = fh.reshape(KCH, 128, NTILE, 128).transpose(1, 2, 0, 3)
        comb[..., 128:256] = fl.reshape(KCH, 128, NTILE, 128).transpose(1, 2, 0, 3)
        in_maps.append({"hf": comb, "wt": wt, "bias": bias, "msk": msk})
    return in_maps


def _run(in_maps, **kwargs):
    nc = _get_nc()
    return run_bass_kernel_spmd(nc, in_maps, core_ids=list(range(NCORES)), **kwargs)


def _assemble(results):
    out = np.empty((NTOK,), np.float32)
    for c in range(NCORES):
        out[c * TPC:(c + 1) * TPC] = results[c]["out"].T.reshape(TPC)
    return out.reshape(B, T)


def kernel(**inputs):
    in_maps = _prep_inputs(**inputs)
    res = _run(in_maps)
    return _assemble(res.results)


def kernel_traced(**inputs):
    in_maps = _prep_inputs(**inputs)
    res = _run(in_maps, trace=True)
    return _assemble(res.results), res
